# revision 18
# baseline (speedup 1.0000x reference)
"""Multi-head self-attention Trainium2 kernel (8 NeuronCores, tensor-parallel over heads).

Problem: x[2,2048,1024], W_qkv[3072,1024], b_qkv[3072], W_out[1024,1024], b_out[1024]
  qkv = x @ W_qkv.T + b_qkv ; per-head attention (16 heads, hd=64) ; out = ctx @ W_out.T + b_out

Sharding: head-parallel. Core c owns heads (2c, 2c+1) for both batches. Each core
computes its 2 heads' Q,K,V (full sequence), attention, and a partial output
projection (columns of W_out for its heads). Host sums the 8 bf16 partials and
adds b_out plus the V-bias fold (W_out @ b_v, a constant row).

On-core dataflow (all matmuls bf16, psums f32):
  - Everything is one long stream of attention "slots" (8 chunks x 18 slots).
    Slot j of a chunk emits scores(j), exp(j-1), AV(j-2): the one-slot skew
    means every exp's input is ready a full slot early, so the Activation
    engine (the ~133us floor) never starves behind PE filler work.
  - QKV projection work (all 4 x-groups) is queued as filler items drained
    between slots, with explicit milestones forcing a block to be emitted
    before the first scores/AV that reads it. No serial head phase.
  - scores transposed: S^T[k, q] = K @ Q^T per head; exp on ScalarE -> E bf16.
  - AV *flipped*: stationary E[k, q-block 128], moving V2[k, 65] (ones column
    appended) -> psC[q, 65] accumulated over k, denominator per-partition in
    col 64. Normalization is a batched reciprocal + tensor_scalar_mul.
  - ctx[q, vd both heads] -> PE transpose -> ctxT[vd, q] -> output projection
    -> bf16 partial DMA'd out. Per-chunk epilogue rides in the next chunk's
    slots; the last chunk's epilogue uses the then-idle Act engine for evac.
"""
import sys
sys.path.insert(0, '/opt/trn_rl_repo')

import numpy as np
import ml_dtypes
from collections import deque
from contextlib import ExitStack

import concourse.bass as bass
import concourse.bacc as bacc
import concourse.tile as tile
from concourse import mybir
from concourse.bass_utils import run_bass_kernel_spmd

F32 = mybir.dt.float32
BF16 = mybir.dt.bfloat16
EXP = mybir.ActivationFunctionType.Exp
BF = ml_dtypes.bfloat16

AVLAG = 4
B, N, D = 2, 2048, 1024
BN = B * N            # 4096
HEADS, HD = 16, 64
NCORES = 8
HPC = HEADS // NCORES  # heads per core = 2
SCALE = 1.0 / np.sqrt(HD)

_cached = {}


def build_nc():
    nc = bacc.Bacc("TRN2", target_bir_lowering=False, debug=False, num_devices=NCORES)
    xT = nc.declare_dram_parameter("xT", [D, BN], BF16, isOutput=False)
    wqkvT = nc.declare_dram_parameter("wqkvT", [D, 384], BF16, isOutput=False)
    bqk = nc.declare_dram_parameter("bqk", [128, 2], F32, isOutput=False)
    woT = nc.declare_dram_parameter("woT", [128, D], BF16, isOutput=False)
    ident = nc.declare_dram_parameter("ident", [128, 128], BF16, isOutput=False)
    out = nc.declare_dram_parameter("out", [BN, D], BF16, isOutput=True)

    with tile.TileContext(nc) as tc, ExitStack() as ctx:
        singles = ctx.enter_context(tc.tile_pool(name="singles", bufs=1))
        wq_sb = singles.tile([128, 8, 384], BF16)   # [d-part, d-tile, (q|k|v)x2h]
        QT = singles.tile([128, BN], BF16)
        KT = singles.tile([128, BN], BF16)
        VT = singles.tile([128, BN], BF16)
        V2 = singles.tile([128, 32, 130], BF16)     # per kb: [k, vd h0 | 1 | vd h1 | 1]
        id_sb = singles.tile([128, 128], BF16)
        woT_sb = singles.tile([128, D], BF16)
        bqk_sb = singles.tile([128, 2], F32)

        nc.gpsimd.memset(V2[:, :, 64:65], 1.0)
        nc.gpsimd.memset(V2[:, :, 129:130], 1.0)

        xpool = ctx.enter_context(tc.tile_pool(name="xg", bufs=3))

        def load_xg(g):
            # two half-group DMAs (d 0-3 / 4-7): few HWDGE slots, and the
            # first qkv matmuls can start after the first half lands
            xg = xpool.tile([128, 8, 1024], BF16, name="xg")
            for half in range(2):
                src_ap = xT[half * 512:(half + 1) * 512,
                            g * 1024:(g + 1) * 1024]
                nc.sync.dma_start(
                    out=xg[:, half * 4:(half + 1) * 4, :],
                    in_=src_ap.rearrange("(d p) c -> p d c", d=4))
            return xg

        nc.sync.dma_start(out=bqk_sb, in_=bqk[:, :])
        xg0 = xpool.tile([128, 8, 1024], BF16, name="xg")
        for half in range(2):
            nc.sync.dma_start(
                out=wq_sb[:, half * 4:(half + 1) * 4, :],
                in_=wqkvT[half * 512:(half + 1) * 512, :]
                .rearrange("(d p) c -> p d c", d=4))
            nc.sync.dma_start(
                out=xg0[:, half * 4:(half + 1) * 4, :],
                in_=xT[half * 512:(half + 1) * 512, 0:1024]
                .rearrange("(d p) c -> p d c", d=4))
        nc.sync.dma_start(out=id_sb, in_=ident[:, :])
        xg1 = load_xg(1)
        nc.sync.dma_start(out=woT_sb, in_=woT[:, :])

        with tc.tile_pool(name="pss", bufs=2, space="PSUM") as pss, \
             tc.tile_pool(name="psc", bufs=1, space="PSUM") as pscp, \
             tc.tile_pool(name="pstt", bufs=1, space="PSUM") as pstt, \
             tc.tile_pool(name="scr", bufs=1, space="PSUM") as scr, \
             tc.tile_pool(name="ep", bufs=10) as epool, \
             tc.tile_pool(name="cs", bufs=8) as cspool, \
             tc.tile_pool(name="ct", bufs=2) as ctpool, \
             tc.tile_pool(name="rc", bufs=4) as rcpool, \
             tc.tile_pool(name="obp", bufs=3) as obpool:

            # [:, 0:128] ctx transposes, [:, 128:256] filler V transposes
            psT = pstt.tile([128, 256], BF16, name="psT")

            # p-state warmup: the PE clock ramps to full speed only after
            # ~3us of continuous execution. The head is DMA-bound anyway, so
            # run back-to-back dummy matmuls so real work starts at 2.4GHz.
            dmy = singles.tile([128, 128], BF16, name="dmy")
            nc.gpsimd.memset(dmy[:, :], 0.5)
            dmy_ps = pss.tile([128, 1024], F32, name="pS")
            for _ in range(45):
                nc.tensor.matmul(dmy_ps[:, 0:128], dmy, dmy,
                                 start=True, stop=True)
            warm = pss.tile([128, 1024], F32, name="pS")  # restore parity
            nc.tensor.matmul(warm[:, 0:128], dmy, dmy, start=True, stop=True)

            fast_q = deque()    # ctx-post items (DVE); never touches scr
            # (fn, cost_ns, is_proj): qkv blocks, V transposes, proj items
            scr_q = deque()
            drained = {"n": 0}
            mile = {}           # milestone key -> required drained count
            enq = {"n": 0}

            def run_next():
                fn, _, _ = scr_q.popleft()
                fn()
                drained["n"] += 1

            def need(key):
                m = mile[key]
                while drained["n"] < m:
                    run_next()

            def enqueue(items, keys=()):
                scr_q.extend(items)
                enq["n"] += len(items)
                for k in keys:
                    mile[k] = enq["n"]

            def vtrans_item(kb):
                def fn():
                    nc.tensor.transpose(psT[:, 128:256],
                                        VT[:, kb * 128:(kb + 1) * 128], id_sb)
                    src = psT[:, 128:256].rearrange("p (t u) -> p t u", t=2)
                    dst = V2[:, kb, :].rearrange("p (t u) -> p t u", t=2)[:, :, 0:64]
                    nc.vector.tensor_copy(dst, src)
                return fn

            def enqueue_block(m, g, nh, xg):
                """One qkv projection block: 4 consecutive scr items of 2
                contraction tiles each (~430ns of PE per item)."""
                st = {}

                def part(d0):
                    def fn():
                        if d0 == 0:
                            st["p"] = scr.tile([128, 512], F32, tag="scr",
                                               name="fps")
                        p = st["p"]
                        for d in range(d0, d0 + 2):
                            nc.tensor.matmul(
                                p, wq_sb[:, d, m * 128:(m + 1) * 128],
                                xg[:, d, nh * 512:(nh + 1) * 512],
                                start=(d == 0), stop=(d == 7))
                        if d0 == 6:
                            cols = bass.ds(g * 1024 + nh * 512, 512)
                            if m == 0:
                                nc.vector.tensor_scalar_add(
                                    QT[:, cols], p, bqk_sb[:, 0:1])
                            elif m == 1:
                                nc.vector.tensor_scalar_add(
                                    KT[:, cols], p, bqk_sb[:, 1:2])
                            else:
                                nc.vector.tensor_copy(VT[:, cols], p)
                            del st["p"]
                    return fn

                enqueue([(part(d0), 430, False) for d0 in (0, 2, 4, 6)],
                        keys=[("QKV"[m], g, nh)])

            def enqueue_vt(g):
                for kb in range(g * 8, g * 8 + 8):
                    enqueue([(vtrans_item(kb), 120, False)],
                            keys=[("vt", kb)])

            def enqueue_kv(g, xg):
                for m in (1, 2):
                    for nh in range(2):
                        enqueue_block(m, g, nh, xg)
                enqueue_vt(g)

            # ---- per-chunk epilogue builders ----
            def make_post(c, psC, store):
                def fn():
                    cs_all = cspool.tile([128, 4, 128], BF16, name="cs")
                    for h in range(2):
                        rec = rcpool.tile([128, 4], F32, name="rec")
                        pC = psC[h].rearrange("p (t u) -> p t u", t=4)
                        nc.vector.reciprocal(rec, pC[:, :, 64:65])
                        nc.vector.tensor_mul(
                            cs_all[:, :, h * 64:(h + 1) * 64],
                            pC[:, :, 0:64],
                            rec.unsqueeze(2).broadcast_to([128, 4, 64]))
                    store["cs"] = cs_all
                return fn

            def make_projs(c, store):
                b, qB = c // 4, c % 4

                def proj_a(qb):
                    def fn():
                        nc.tensor.transpose(psT[:, 0:128],
                                            store["cs"][:, qb, :], id_sb)
                        ct = ctpool.tile([128, 128], BF16, name="ct")
                        nc.vector.tensor_copy(ct, psT[:, 0:128])
                        po = scr.tile([128, 512], F32, tag="scr", name="po")
                        nc.tensor.matmul(po, ct, woT_sb[:, 0:512],
                                         start=True, stop=True)
                        ob = obpool.tile([128, 1024], BF16, name="ob")
                        nc.vector.tensor_copy(ob[:, 0:512], po)
                        store[(qb, "ct")] = ct
                        store[(qb, "ob")] = ob
                    return fn

                def proj_b(qb):
                    def fn():
                        ct = store.pop((qb, "ct"))
                        ob = store.pop((qb, "ob"))
                        po = scr.tile([128, 512], F32, tag="scr", name="po")
                        nc.tensor.matmul(po, ct, woT_sb[:, 512:1024],
                                         start=True, stop=True)
                        nc.vector.tensor_copy(ob[:, 512:1024], po)
                        rows = bass.ds(b * N + qB * 512 + qb * 128, 128)
                        nc.sync.dma_start(out=out[rows, :], in_=ob)
                    return fn

                return [((proj_a(qb) if k == 0 else proj_b(qb)),
                         280 if k == 0 else 230, True)
                        for qb in range(4) for k in range(2)]

            # ---- one flat stream of 130 global slots over 8 chunks ----
            # slot t: scores(t), exp(t-1), AV(t-2). Every engine's waits
            # cover its full emission-order prefix, so the Act engine keeps
            # up only if PE work between consecutive exps stays ~<=1us:
            # fillers are small items paced by a per-slot cost budget.
            psC_c = {}
            store_c = {}
            E_h = {}
            pS_h = {}
            xgs = {0: xg0, 1: xg1}
            enqueue_block(1, 0, 0, xg0)       # K g0 nh0
            enqueue_block(0, 0, 0, xg0)       # Q g0 nh0
            enqueue_block(1, 0, 1, xg0)       # K g0 nh1
            for nh in range(2):
                enqueue_block(2, 0, nh, xg0)  # V g0
            enqueue_vt(0)
            enqueue_kv(1, xg1)
            for t in range(128 + AVLAG):
                if t == 2:
                    enqueue_block(0, 0, 1, xg0)     # Q g0 nh1 (chunk (0,1))
                    enqueue_block(0, 1, 0, xg1)
                    enqueue_block(0, 1, 1, xg1)
                elif t == 4:
                    xgs[2] = load_xg(2)
                    enqueue_kv(2, xgs[2])
                elif t == 18:
                    xgs[3] = load_xg(3)
                    enqueue_kv(3, xgs[3])
                elif t == 32:
                    enqueue_block(0, 2, 0, xgs[2])
                    enqueue_block(0, 2, 1, xgs[2])
                elif t == 48:
                    enqueue_block(0, 3, 0, xgs[3])
                    enqueue_block(0, 3, 1, xgs[3])

                for _ in range(2):
                    if fast_q:
                        fast_q.popleft()()

                if t < 128:
                    c, j = t // 16, t % 16
                    b, qB = c // 4, c % 4
                    kb32 = b * 16 + j
                    need(("K", kb32 // 8, (kb32 % 8) // 4))
                    need(("Q", (b * 2048 + qB * 512) // 1024,
                          ((b * 2048 + qB * 512) % 1024) // 512))
                    qs = bass.ds(b * N + qB * 512, 512)
                    ks = bass.ts(kb32, 128)
                    pS = pss.tile([128, 1024], F32, name="pS")
                    nc.tensor.matmul(pS[:, 0:512], KT[0:64, ks],
                                     QT[0:64, qs], start=True, stop=True)
                    nc.tensor.matmul(pS[:, 512:1024], KT[64:128, ks],
                                     QT[64:128, qs], start=True, stop=True)
                    pS_h[t] = pS
                if 1 <= t <= 128:
                    E = epool.tile([128, 1024], BF16, name="E")
                    nc.scalar.activation(E, pS_h.pop(t - 1), EXP,
                                         scale=float(SCALE))
                    E_h[t - 1] = E
                if t >= AVLAG:
                    e = t - AVLAG
                    c, kb = e // 16, e % 16
                    b, qB = c // 4, c % 4
                    kb32 = b * 16 + kb
                    need(("vt", kb32))
                    if kb == 0:
                        # allocated after chunk c-1's ctx-post was emitted
                        # (fast_q drain above). Accumulate with start=False
                        # onto zeroed banks: a start=True zeroes the whole
                        # bank, wiping sibling accumulators.
                        psC = psC_c[c] = (
                            pscp.tile([128, 512], F32, tag="psca", name="psCa"),
                            pscp.tile([128, 512], F32, tag="pscb", name="psCb"))
                        for h in range(2):
                            z = psC[h].rearrange("p (t u) -> p t u", t=4)
                            nc.vector.memset(z[:, :, 0:65], 0.0)
                    psC = psC_c[c]
                    Ep = E_h.pop(e)
                    for h in range(2):
                        for qb in range(4):
                            nc.tensor.matmul(
                                psC[h][:, qb * 128:qb * 128 + 65],
                                Ep[:, h * 512 + qb * 128:
                                   h * 512 + (qb + 1) * 128],
                                V2[:, kb32, h * 65:(h + 1) * 65],
                                start=False, stop=(kb == 15),
                                skip_group_check=True)
                    if kb == 15:
                        store_c[c] = {}
                        make_post(c, psC, store_c[c])()
                        if c < 7:
                            enqueue(make_projs(c, store_c[c]))

                # cost-budgeted pacing (at most one proj item per slot
                # unless the backlog is deep)
                backlog = len(scr_q)
                budget = 560 if backlog > 24 else 520
                projs_left = 1
                while scr_q:
                    fn, cost, isp = scr_q[0]
                    if cost > budget or (isp and not projs_left):
                        break
                    budget -= cost
                    if isp:
                        projs_left -= 1
                    run_next()

            # ---- tail: chunk 7 epilogue with the now-idle Act engine ----
            store = store_c[7]
            b, qB = 1, 3
            for qb in range(4):
                nc.tensor.transpose(psT[:, 0:128], store["cs"][:, qb, :],
                                    id_sb)
                ct = ctpool.tile([128, 128], BF16, name="ct")
                nc.scalar.copy(ct, psT[:, 0:128])
                po = pss.tile([128, 1024], F32, name="pS")
                nc.tensor.matmul(po[:, 0:512], ct, woT_sb[:, 0:512],
                                 start=True, stop=True)
                nc.tensor.matmul(po[:, 512:1024], ct, woT_sb[:, 512:1024],
                                 start=True, stop=True)
                ob = obpool.tile([128, 1024], BF16, name="ob")
                nc.scalar.copy(ob[:, 0:512], po[:, 0:512])
                nc.vector.tensor_copy(ob[:, 512:1024], po[:, 512:1024])
                rows = bass.ds(b * N + qB * 512 + qb * 128, 128)
                nc.sync.dma_start(out=out[rows, :], in_=ob)
            while scr_q:
                run_next()
            while fast_q:
                fast_q.popleft()()

    nc.compile()
    return nc


def _host_prep(x, W_qkv, b_qkv, W_out):
    x2 = x.reshape(BN, D).T.astype(BF)                 # [D, BN]
    ident = np.eye(128, dtype=np.float32).astype(BF)
    in_maps = []
    for c in range(NCORES):
        lo = HPC * c * HD                              # first ctx dim of this core
        rows = np.concatenate([np.arange(m * D + lo, m * D + lo + 128)
                               for m in range(3)])
        wqkvT = np.ascontiguousarray(W_qkv[rows, :].T).astype(BF)   # [1024, 384]
        bqk2 = np.stack([b_qkv[lo:lo + 128],
                         b_qkv[D + lo:D + lo + 128]], axis=1).astype(np.float32)
        bqk2 = np.ascontiguousarray(bqk2)
        woT = np.ascontiguousarray(W_out[:, lo:lo + 128].T).astype(BF)  # [128, 1024]
        in_maps.append({
            "xT": x2, "wqkvT": wqkvT, "bqk": bqk2, "woT": woT, "ident": ident,
        })
    return in_maps


def kernel(x, W_qkv, b_qkv, W_out, b_out, _trace=False):
    x = np.asarray(x, dtype=np.float32)
    W_qkv = np.asarray(W_qkv, dtype=np.float32)
    b_qkv = np.asarray(b_qkv, dtype=np.float32)
    W_out = np.asarray(W_out, dtype=np.float32)
    b_out = np.asarray(b_out, dtype=np.float32)

    if "nc" not in _cached:
        _cached["nc"] = build_nc()
    nc = _cached["nc"]

    in_maps = _host_prep(x, W_qkv, b_qkv, W_out)
    res = run_bass_kernel_spmd(nc, in_maps, list(range(NCORES)), trace=_trace)
    _cached["last_result"] = res

    total = np.zeros((BN, D), dtype=np.float64)
    for c in range(NCORES):
        total += res.results[c]["out"].astype(np.float64)
    # V bias never went to the device: ctx bias b_v contributes the constant
    # row b_v @ W_out.T = W_out @ b_v to every output row.
    total += b_out.astype(np.float64)
    total += W_out.astype(np.float64) @ b_qkv[2 * D:3 * D].astype(np.float64)
    return total.reshape(B, N, D).astype(np.float32)


if __name__ == "__main__":
    rng = np.random.default_rng(0)
    x = rng.standard_normal((B, N, D), dtype=np.float32)
    s = 1.0 / np.sqrt(D)
    W_qkv = rng.uniform(-s, s, (3 * D, D)).astype(np.float32)
    b_qkv = rng.uniform(-s, s, (3 * D,)).astype(np.float32)
    W_out = rng.uniform(-s, s, (D, D)).astype(np.float32)
    b_out = rng.uniform(-s, s, (D,)).astype(np.float32)
    got = kernel(x, W_qkv, b_qkv, W_out, b_out)
    print("kernel ran, out shape", got.shape)


# revision 20
# speedup vs baseline: 1.0154x; 1.0154x over previous
"""Multi-head self-attention Trainium2 kernel (8 NeuronCores, tensor-parallel over heads).

Problem: x[2,2048,1024], W_qkv[3072,1024], b_qkv[3072], W_out[1024,1024], b_out[1024]
  qkv = x @ W_qkv.T + b_qkv ; per-head attention (16 heads, hd=64) ; out = ctx @ W_out.T + b_out

Sharding: head-parallel. Core c owns heads (2c, 2c+1) for both batches. Each core
computes its 2 heads' Q,K,V (full sequence), attention, and a partial output
projection (columns of W_out for its heads). Host sums the 8 bf16 partials and
adds b_out plus the V-bias fold (W_out @ b_v, a constant row).

On-core dataflow (all matmuls bf16, psums f32):
  - Everything is one long stream of attention "slots" (8 chunks x 18 slots).
    Slot j of a chunk emits scores(j), exp(j-1), AV(j-2): the one-slot skew
    means every exp's input is ready a full slot early, so the Activation
    engine (the ~133us floor) never starves behind PE filler work.
  - QKV projection work (all 4 x-groups) is queued as filler items drained
    between slots, with explicit milestones forcing a block to be emitted
    before the first scores/AV that reads it. No serial head phase.
  - scores transposed: S^T[k, q] = K @ Q^T per head; exp on ScalarE -> E bf16.
  - AV *flipped*: stationary E[k, q-block 128], moving V2[k, 65] (ones column
    appended) -> psC[q, 65] accumulated over k, denominator per-partition in
    col 64. Normalization is a batched reciprocal + tensor_scalar_mul.
  - ctx[q, vd both heads] -> PE transpose -> ctxT[vd, q] -> output projection
    -> bf16 partial DMA'd out. Per-chunk epilogue rides in the next chunk's
    slots; the last chunk's epilogue uses the then-idle Act engine for evac.
"""
import sys
sys.path.insert(0, '/opt/trn_rl_repo')

import numpy as np
import ml_dtypes
from collections import deque
from contextlib import ExitStack

import concourse.bass as bass
import concourse.bacc as bacc
import concourse.tile as tile
from concourse import mybir
from concourse.bass_utils import run_bass_kernel_spmd

F32 = mybir.dt.float32
BF16 = mybir.dt.bfloat16
EXP = mybir.ActivationFunctionType.Exp
BF = ml_dtypes.bfloat16

AVLAG = 2
B, N, D = 2, 2048, 1024
BN = B * N            # 4096
HEADS, HD = 16, 64
NCORES = 8
HPC = HEADS // NCORES  # heads per core = 2
SCALE = 1.0 / np.sqrt(HD)

_cached = {}


def build_nc():
    nc = bacc.Bacc("TRN2", target_bir_lowering=False, debug=False, num_devices=NCORES)
    xT = nc.declare_dram_parameter("xT", [D, BN], BF16, isOutput=False)
    wqkvT = nc.declare_dram_parameter("wqkvT", [D, 384], BF16, isOutput=False)
    bqk = nc.declare_dram_parameter("bqk", [128, 2], F32, isOutput=False)
    woT = nc.declare_dram_parameter("woT", [128, D], BF16, isOutput=False)
    ident = nc.declare_dram_parameter("ident", [128, 128], BF16, isOutput=False)
    out = nc.declare_dram_parameter("out", [BN, D], BF16, isOutput=True)

    with tile.TileContext(nc) as tc, ExitStack() as ctx:
        singles = ctx.enter_context(tc.tile_pool(name="singles", bufs=1))
        wq_sb = singles.tile([128, 8, 384], BF16)   # [d-part, d-tile, (q|k|v)x2h]
        QT = singles.tile([128, BN], BF16)
        KT = singles.tile([128, BN], BF16)
        VT = singles.tile([128, BN], BF16)
        V2 = singles.tile([128, 32, 130], BF16)     # per kb: [k, vd h0 | 1 | vd h1 | 1]
        id_sb = singles.tile([128, 128], BF16)
        woT_sb = singles.tile([128, D], BF16)
        bqk_sb = singles.tile([128, 2], F32)

        nc.gpsimd.memset(V2[:, :, 64:65], 1.0)
        nc.gpsimd.memset(V2[:, :, 129:130], 1.0)

        xpool = ctx.enter_context(tc.tile_pool(name="xg", bufs=3))

        def load_xg(g):
            # two half-group DMAs (d 0-3 / 4-7): few HWDGE slots, and the
            # first qkv matmuls can start after the first half lands
            xg = xpool.tile([128, 8, 1024], BF16, name="xg")
            for half in range(2):
                src_ap = xT[half * 512:(half + 1) * 512,
                            g * 1024:(g + 1) * 1024]
                nc.sync.dma_start(
                    out=xg[:, half * 4:(half + 1) * 4, :],
                    in_=src_ap.rearrange("(d p) c -> p d c", d=4))
            return xg

        nc.sync.dma_start(out=bqk_sb, in_=bqk[:, :])
        xg0 = xpool.tile([128, 8, 1024], BF16, name="xg")
        for half in range(2):
            nc.sync.dma_start(
                out=wq_sb[:, half * 4:(half + 1) * 4, :],
                in_=wqkvT[half * 512:(half + 1) * 512, :]
                .rearrange("(d p) c -> p d c", d=4))
            nc.sync.dma_start(
                out=xg0[:, half * 4:(half + 1) * 4, :],
                in_=xT[half * 512:(half + 1) * 512, 0:1024]
                .rearrange("(d p) c -> p d c", d=4))
        nc.sync.dma_start(out=id_sb, in_=ident[:, :])
        xg1 = load_xg(1)
        nc.sync.dma_start(out=woT_sb, in_=woT[:, :])

        with tc.tile_pool(name="pss", bufs=2, space="PSUM") as pss, \
             tc.tile_pool(name="psc", bufs=1, space="PSUM") as pscp, \
             tc.tile_pool(name="pstt", bufs=1, space="PSUM") as pstt, \
             tc.tile_pool(name="scr", bufs=1, space="PSUM") as scr, \
             tc.tile_pool(name="ep", bufs=10) as epool, \
             tc.tile_pool(name="cs", bufs=8) as cspool, \
             tc.tile_pool(name="ct", bufs=2) as ctpool, \
             tc.tile_pool(name="rc", bufs=4) as rcpool, \
             tc.tile_pool(name="obp", bufs=3) as obpool:

            # [:, 0:128] ctx transposes, [:, 128:256] filler V transposes
            psT = pstt.tile([128, 256], BF16, name="psT")

            # p-state warmup: the PE clock ramps to full speed only after
            # ~3us of continuous execution. The head is DMA-bound anyway, so
            # run back-to-back dummy matmuls so real work starts at 2.4GHz.
            dmy = singles.tile([128, 128], BF16, name="dmy")
            nc.gpsimd.memset(dmy[:, :], 0.5)
            dmy_ps = pss.tile([128, 1024], F32, name="pS")
            for _ in range(45):
                nc.tensor.matmul(dmy_ps[:, 0:128], dmy, dmy,
                                 start=True, stop=True)
            warm = pss.tile([128, 1024], F32, name="pS")  # restore parity
            nc.tensor.matmul(warm[:, 0:128], dmy, dmy, start=True, stop=True)

            fast_q = deque()    # ctx-post items (DVE); never touches scr
            # (fn, cost_ns, is_proj): qkv blocks, V transposes, proj items
            scr_q = deque()
            drained = {"n": 0}
            mile = {}           # milestone key -> required drained count
            enq = {"n": 0}

            def run_next():
                fn, _, _ = scr_q.popleft()
                fn()
                drained["n"] += 1

            def need(key):
                m = mile[key]
                while drained["n"] < m:
                    run_next()

            def enqueue(items, keys=()):
                scr_q.extend(items)
                enq["n"] += len(items)
                for k in keys:
                    mile[k] = enq["n"]

            def vtrans_item(kb):
                def fn():
                    nc.tensor.transpose(psT[:, 128:256],
                                        VT[:, kb * 128:(kb + 1) * 128], id_sb)
                    src = psT[:, 128:256].rearrange("p (t u) -> p t u", t=2)
                    dst = V2[:, kb, :].rearrange("p (t u) -> p t u", t=2)[:, :, 0:64]
                    nc.vector.tensor_copy(dst, src)
                return fn

            def enqueue_block(m, g, nh, xg):
                """One qkv projection block: 4 consecutive scr items of 2
                contraction tiles each (~430ns of PE per item)."""
                st = {}

                def part(d0):
                    def fn():
                        if d0 == 0:
                            st["p"] = scr.tile([128, 512], F32, tag="scr",
                                               name="fps")
                        p = st["p"]
                        for d in range(d0, d0 + 2):
                            nc.tensor.matmul(
                                p, wq_sb[:, d, m * 128:(m + 1) * 128],
                                xg[:, d, nh * 512:(nh + 1) * 512],
                                start=(d == 0), stop=(d == 7))
                        if d0 == 6:
                            cols = bass.ds(g * 1024 + nh * 512, 512)
                            if m == 0:
                                nc.vector.tensor_scalar_add(
                                    QT[:, cols], p, bqk_sb[:, 0:1])
                            elif m == 1:
                                nc.vector.tensor_scalar_add(
                                    KT[:, cols], p, bqk_sb[:, 1:2])
                            else:
                                nc.vector.tensor_copy(VT[:, cols], p)
                            del st["p"]
                    return fn

                enqueue([(part(d0), 430, False) for d0 in (0, 2, 4, 6)],
                        keys=[("QKV"[m], g, nh)])

            def enqueue_vt(g):
                for kb in range(g * 8, g * 8 + 8):
                    enqueue([(vtrans_item(kb), 120, False)],
                            keys=[("vt", kb)])

            def enqueue_kv(g, xg):
                for m in (1, 2):
                    for nh in range(2):
                        enqueue_block(m, g, nh, xg)
                enqueue_vt(g)

            # ---- per-chunk epilogue builders ----
            def make_post(c, psC, store):
                def fn():
                    cs_all = cspool.tile([128, 4, 128], BF16, name="cs")
                    for h in range(2):
                        rec = rcpool.tile([128, 4], F32, name="rec")
                        pC = psC[h].rearrange("p (t u) -> p t u", t=4)
                        nc.vector.reciprocal(rec, pC[:, :, 64:65])
                        nc.vector.tensor_mul(
                            cs_all[:, :, h * 64:(h + 1) * 64],
                            pC[:, :, 0:64],
                            rec.unsqueeze(2).broadcast_to([128, 4, 64]))
                    store["cs"] = cs_all
                return fn

            def make_projs(c, store):
                b, qB = c // 4, c % 4

                def proj_a(qb):
                    def fn():
                        nc.tensor.transpose(psT[:, 0:128],
                                            store["cs"][:, qb, :], id_sb)
                        ct = ctpool.tile([128, 128], BF16, name="ct")
                        nc.vector.tensor_copy(ct, psT[:, 0:128])
                        po = scr.tile([128, 512], F32, tag="scr", name="po")
                        nc.tensor.matmul(po, ct, woT_sb[:, 0:512],
                                         start=True, stop=True)
                        ob = obpool.tile([128, 1024], BF16, name="ob")
                        nc.vector.tensor_copy(ob[:, 0:512], po)
                        store[(qb, "ct")] = ct
                        store[(qb, "ob")] = ob
                    return fn

                def proj_b(qb):
                    def fn():
                        ct = store.pop((qb, "ct"))
                        ob = store.pop((qb, "ob"))
                        po = scr.tile([128, 512], F32, tag="scr", name="po")
                        nc.tensor.matmul(po, ct, woT_sb[:, 512:1024],
                                         start=True, stop=True)
                        nc.vector.tensor_copy(ob[:, 512:1024], po)
                        rows = bass.ds(b * N + qB * 512 + qb * 128, 128)
                        nc.sync.dma_start(out=out[rows, :], in_=ob)
                    return fn

                return [((proj_a(qb) if k == 0 else proj_b(qb)),
                         280 if k == 0 else 230, True)
                        for qb in range(4) for k in range(2)]

            # ---- one flat stream of 130 global slots over 8 chunks ----
            # slot t: scores(t), exp(t-1), AV(t-2). Every engine's waits
            # cover its full emission-order prefix, so the Act engine keeps
            # up only if PE work between consecutive exps stays ~<=1us:
            # fillers are small items paced by a per-slot cost budget.
            psC_c = {}
            store_c = {}
            E_h = {}
            pS_h = {}
            xgs = {0: xg0, 1: xg1}
            # batch-0 qkv work runs entirely before the stream: the head is
            # DMA-bound anyway, and keeping it out of the exp-covered stream
            # keeps every exp's PE prefix free of DMA-gated stalls
            enqueue_block(1, 0, 0, xg0)       # K g0 nh0
            enqueue_block(0, 0, 0, xg0)       # Q g0 nh0
            enqueue_block(1, 0, 1, xg0)       # K g0 nh1
            for nh in range(2):
                enqueue_block(2, 0, nh, xg0)  # V g0
            enqueue_vt(0)
            enqueue_kv(1, xg1)
            enqueue_block(0, 0, 1, xg0)       # Q g0 nh1
            enqueue_block(0, 1, 0, xg1)
            enqueue_block(0, 1, 1, xg1)
            while scr_q:
                run_next()
            for t in range(128 + AVLAG):
                if t == 2:
                    xgs[2] = load_xg(2)
                    enqueue_kv(2, xgs[2])
                elif t == 4:
                    xgs[3] = load_xg(3)
                    enqueue_kv(3, xgs[3])
                elif t == 24:
                    enqueue_block(0, 2, 0, xgs[2])
                    enqueue_block(0, 2, 1, xgs[2])
                elif t == 40:
                    enqueue_block(0, 3, 0, xgs[3])
                    enqueue_block(0, 3, 1, xgs[3])

                for _ in range(2):
                    if fast_q:
                        fast_q.popleft()()

                if t < 128:
                    c, j = t // 16, t % 16
                    b, qB = c // 4, c % 4
                    kb32 = b * 16 + j
                    need(("K", kb32 // 8, (kb32 % 8) // 4))
                    need(("Q", (b * 2048 + qB * 512) // 1024,
                          ((b * 2048 + qB * 512) % 1024) // 512))
                    qs = bass.ds(b * N + qB * 512, 512)
                    ks = bass.ts(kb32, 128)
                    pS = pss.tile([128, 1024], F32, name="pS")
                    nc.tensor.matmul(pS[:, 0:512], KT[0:64, ks],
                                     QT[0:64, qs], start=True, stop=True)
                    nc.tensor.matmul(pS[:, 512:1024], KT[64:128, ks],
                                     QT[64:128, qs], start=True, stop=True)
                    pS_h[t] = pS
                if 1 <= t <= 128:
                    E = epool.tile([128, 1024], BF16, name="E")
                    nc.scalar.activation(E, pS_h.pop(t - 1), EXP,
                                         scale=float(SCALE))
                    E_h[t - 1] = E
                if t >= AVLAG:
                    e = t - AVLAG
                    c, kb = e // 16, e % 16
                    b, qB = c // 4, c % 4
                    kb32 = b * 16 + kb
                    need(("vt", kb32))
                    if kb == 0:
                        # allocated after chunk c-1's ctx-post was emitted
                        # (fast_q drain above). Accumulate with start=False
                        # onto zeroed banks: a start=True zeroes the whole
                        # bank, wiping sibling accumulators.
                        psC = psC_c[c] = (
                            pscp.tile([128, 512], F32, tag="psca", name="psCa"),
                            pscp.tile([128, 512], F32, tag="pscb", name="psCb"))
                        for h in range(2):
                            z = psC[h].rearrange("p (t u) -> p t u", t=4)
                            nc.vector.memset(z[:, :, 0:65], 0.0)
                    psC = psC_c[c]
                    Ep = E_h.pop(e)
                    for h in range(2):
                        for qb in range(4):
                            nc.tensor.matmul(
                                psC[h][:, qb * 128:qb * 128 + 65],
                                Ep[:, h * 512 + qb * 128:
                                   h * 512 + (qb + 1) * 128],
                                V2[:, kb32, h * 65:(h + 1) * 65],
                                start=False, stop=(kb == 15),
                                skip_group_check=True)
                    if kb == 15:
                        store_c[c] = {}
                        make_post(c, psC, store_c[c])()
                        if c < 7:
                            enqueue(make_projs(c, store_c[c]))

                # cost-budgeted pacing (at most one proj item per slot
                # unless the backlog is deep)
                backlog = len(scr_q)
                budget = 560 if backlog > 24 else 520
                projs_left = 1
                while scr_q:
                    fn, cost, isp = scr_q[0]
                    if cost > budget or (isp and not projs_left):
                        break
                    budget -= cost
                    if isp:
                        projs_left -= 1
                    run_next()

            # ---- tail: chunk 7 epilogue with the now-idle Act engine ----
            store = store_c[7]
            b, qB = 1, 3
            for qb in range(4):
                nc.tensor.transpose(psT[:, 0:128], store["cs"][:, qb, :],
                                    id_sb)
                ct = ctpool.tile([128, 128], BF16, name="ct")
                nc.scalar.copy(ct, psT[:, 0:128])
                po = pss.tile([128, 1024], F32, name="pS")
                nc.tensor.matmul(po[:, 0:512], ct, woT_sb[:, 0:512],
                                 start=True, stop=True)
                nc.tensor.matmul(po[:, 512:1024], ct, woT_sb[:, 512:1024],
                                 start=True, stop=True)
                ob = obpool.tile([128, 1024], BF16, name="ob")
                nc.scalar.copy(ob[:, 0:512], po[:, 0:512])
                nc.vector.tensor_copy(ob[:, 512:1024], po[:, 512:1024])
                rows = bass.ds(b * N + qB * 512 + qb * 128, 128)
                nc.sync.dma_start(out=out[rows, :], in_=ob)
            while scr_q:
                run_next()
            while fast_q:
                fast_q.popleft()()

    nc.compile()
    return nc


def _host_prep(x, W_qkv, b_qkv, W_out):
    x2 = x.reshape(BN, D).T.astype(BF)                 # [D, BN]
    ident = np.eye(128, dtype=np.float32).astype(BF)
    in_maps = []
    for c in range(NCORES):
        lo = HPC * c * HD                              # first ctx dim of this core
        rows = np.concatenate([np.arange(m * D + lo, m * D + lo + 128)
                               for m in range(3)])
        wqkvT = np.ascontiguousarray(W_qkv[rows, :].T).astype(BF)   # [1024, 384]
        bqk2 = np.stack([b_qkv[lo:lo + 128],
                         b_qkv[D + lo:D + lo + 128]], axis=1).astype(np.float32)
        bqk2 = np.ascontiguousarray(bqk2)
        woT = np.ascontiguousarray(W_out[:, lo:lo + 128].T).astype(BF)  # [128, 1024]
        in_maps.append({
            "xT": x2, "wqkvT": wqkvT, "bqk": bqk2, "woT": woT, "ident": ident,
        })
    return in_maps


def kernel(x, W_qkv, b_qkv, W_out, b_out, _trace=False):
    x = np.asarray(x, dtype=np.float32)
    W_qkv = np.asarray(W_qkv, dtype=np.float32)
    b_qkv = np.asarray(b_qkv, dtype=np.float32)
    W_out = np.asarray(W_out, dtype=np.float32)
    b_out = np.asarray(b_out, dtype=np.float32)

    if "nc" not in _cached:
        _cached["nc"] = build_nc()
    nc = _cached["nc"]

    in_maps = _host_prep(x, W_qkv, b_qkv, W_out)
    res = run_bass_kernel_spmd(nc, in_maps, list(range(NCORES)), trace=_trace)
    _cached["last_result"] = res

    total = np.zeros((BN, D), dtype=np.float64)
    for c in range(NCORES):
        total += res.results[c]["out"].astype(np.float64)
    # V bias never went to the device: ctx bias b_v contributes the constant
    # row b_v @ W_out.T = W_out @ b_v to every output row.
    total += b_out.astype(np.float64)
    total += W_out.astype(np.float64) @ b_qkv[2 * D:3 * D].astype(np.float64)
    return total.reshape(B, N, D).astype(np.float32)


if __name__ == "__main__":
    rng = np.random.default_rng(0)
    x = rng.standard_normal((B, N, D), dtype=np.float32)
    s = 1.0 / np.sqrt(D)
    W_qkv = rng.uniform(-s, s, (3 * D, D)).astype(np.float32)
    b_qkv = rng.uniform(-s, s, (3 * D,)).astype(np.float32)
    W_out = rng.uniform(-s, s, (D, D)).astype(np.float32)
    b_out = rng.uniform(-s, s, (D,)).astype(np.float32)
    got = kernel(x, W_qkv, b_qkv, W_out, b_out)
    print("kernel ran, out shape", got.shape)
